# revision 37
# baseline (speedup 1.0000x reference)
"""Trainium2 Bass kernel for nn_MemoryModel (scatter_memory, 8 cores).

Math (per stage): the Gauss-Legendre quadrature over matrix polynomials
collapses algebraically:

  LHS_k = I - REG*t_k*D + REG^2*(t_k*D@L + t_k^2/2*D@D)      (D=delta_L, L=L_agg)
  integral = sum_k (LHS_k @ (w_k*V)) * exp(dA*t_k)
           = V*S0 - REG*U*S1 + REG^2*P*S1 + REG^2/2*Q*S2
  with V = X - REG*(L@X),  U = D@V, W1 = L@V, P = D@W1, Q = D@U
  and moments S_j = sum_k w_k t_k^j exp(dA t_k)   (elementwise [n,H])
  As_bar @ M = M - REG*(D@M) + REG^2*(D@(L@M)) + REG^2/2*(D@(D@M))

The reference's 8-point rule is evaluated with a 4-point rule here: the
integrand is exp(a t)*poly2(t), a in (-6,0), where |GL4-GL8| < 6e-5 -
far below the accuracy gate.

Distribution: NO collectives (the runtime's collective stack has a 30-95us
variable startup barrier that dwarfs the math). Stage 1 is REPLICATED at
full H=128 on every core; the stage-1/2 handoff is 8 on-chip PE transposes
of u2 = zt + gelu(c1). Stage 2 is H-column-sharded 8 ways (16 cols/core);
outputs are the c2 shards plus the full c1 (host takes core 0's copy).

Scalar weights (-REG, REG^2, ...) are folded into the activation-Copy
pass callbacks (free), so the combine is pure tensor_tensor spread across
the Vector and Pool engines. rhs groups live in one tile R0 = [X|M|Y1|V]
so chained passes slice contiguous column ranges without extra copies.
"""
import os
import sys

import numpy as np

for _p in ("/opt/trn_rl_repo", "/root/.axon_site/_ro/trn_rl_repo"):
    if os.path.isdir(_p) and _p not in sys.path:
        sys.path.insert(0, _p)

import ml_dtypes  # noqa: E402
import concourse.bass as bass  # noqa: E402
import concourse.bacc as bacc  # noqa: E402
import concourse.mybir as mybir  # noqa: E402
import concourse.tile as tile  # noqa: E402
from concourse.bass_utils import run_bass_kernel_spmd  # noqa: E402

F32 = mybir.dt.float32
BF16 = mybir.dt.bfloat16
I32 = mybir.dt.int32
AF = mybir.ActivationFunctionType
OP = mybir.AluOpType
BF = ml_dtypes.bfloat16

NA, H, DIN, E, NN, ED = 1024, 128, 172, 256, 100000, 1
KD = DIN + 2 * ED  # 174
KDP = 256  # padded contraction for the tune matmul
REG = 0.1
REG2 = REG * REG
NCORES = 8
HS = 16  # stage-2 H columns per core
NQ = 8  # node tiles (1024/128)
MW = H + HS  # gather row width: m1 full | m2 shard
NK = 4  # quadrature points (4-pt GL matches the 8-pt reference to <6e-5)

_gl4_nodes = [-0.3399810435848563, 0.3399810435848563,
              -0.8611363115940526, 0.8611363115940526]
_gl4_w = [0.6521451548625461, 0.6521451548625461,
          0.3478548451374538, 0.3478548451374538]
T_NODES = [0.5 * (x + 1.0) for x in _gl4_nodes]
T_W = [0.5 * w for w in _gl4_w]

# f32 const-group free offsets: btune|rms1|rms2|bb1(129)|bb2(17)|actb(9)|ident
F_BT, F_R1, F_R2, F_BB1, F_BB2, F_ACT, F_ID = 0, 1, 2, 3, 132, 149, 158
F_TOT = 286
# bf16 const-group free offsets: wt0|wt1|wb1(129)|wb2(17)|ones
B_WT0, B_WT1, B_WB1, B_WB2, B_ONE = 0, 128, 256, 385, 402
B_TOT = 403

_BUILD_CACHE = {}


def _pin_act_table_set():
    """Restrict walrus's ACT-table choice to natural_log_exp_and_others so
    the kernel's exp/ln mix never ping-pongs table loads."""
    if os.environ.get("BASS_ACT_ROOT_JSON_PATH"):
        return
    try:
        import glob
        import json
        import tempfile

        import neuronxcc

        pwp = os.path.join(os.path.dirname(neuronxcc.__file__), "pwp",
                           "pwp_bin_trainium")
        info = json.load(open(os.path.join(pwp, "act_info.json")))
        keep = [s for s in info["act_func_sets"]
                if s["name"] == "natural_log_exp_and_others"]
        if not keep:
            return
        d = tempfile.mkdtemp(prefix="act_root_")
        for f in glob.glob(os.path.join(pwp, "*")):
            dst = os.path.join(d, os.path.basename(f))
            if not os.path.exists(dst):
                os.symlink(f, dst)
        out = dict(info)
        out["act_func_sets"] = keep
        patched = os.path.join(d, "act_info.json")
        os.unlink(patched)
        with open(patched, "w") as fh:
            json.dump(out, fh)
        import concourse.hw_specs as hw_specs

        tables = {
            keep[0]["name"]: {AF.from_pwp(v) for v in keep[0]["act"].keys()}
        }

        def _tables(arch, _t=tables):
            return _t

        hw_specs.get_activation_tables = _tables
        bacc.get_activation_tables = _tables
        os.environ["BASS_ACT_ROOT_JSON_PATH"] = patched
    except Exception:
        pass


def _heavy_pass(nc, psum, op_sb, ncols, out_cb, rhs_cols, qs=range(NQ)):
    """out = Op @ X: Op is a q-chunked lhsT sbuf tile [128, 8q, 8k, 128]
    (bf16); rhs_cols(k) gives the [128, ncols] bf16 rhs k-tile. Calls
    out_cb(q, ps[128, ncols])."""
    for q in qs:
        ps = psum.tile([128, ncols], F32, tag="hv")
        for k in range(NQ):
            nc.tensor.matmul(
                ps[:],
                lhsT=op_sb[:, q, k, :],
                rhs=rhs_cols(k),
                start=(k == 0),
                stop=(k == NQ - 1),
            )
        out_cb(q, ps)


def _moments(nc, wk, s, W, dA, actb_v):
    """Quadrature moments S_j = sum_k w_k t_k^j exp(dA t_k), j=0..2 (bf16,
    4-point rule)."""
    S0 = wk.tile([128, NQ, W], BF16, tag=f"S0{s}")
    S1 = wk.tile([128, NQ, W], BF16, tag=f"S1{s}")
    S2 = wk.tile([128, NQ, W], BF16, tag=f"S2{s}")
    for k in range(NK):
        tk = float(T_NODES[k])
        wE = wk.tile([128, NQ, W], BF16, tag=f"wE{s}_{k % 2}",
                     name=f"wE{s}_{k}")
        nc.scalar.activation(wE[:], dA[:], AF.Exp, scale=tk,
                             bias=actb_v[:, k + 1:k + 2])
        if k == 0:
            nc.vector.tensor_copy(out=S0[:], in_=wE[:])
            nc.vector.tensor_scalar(out=S1[:], in0=wE[:], scalar1=tk,
                                    scalar2=None, op0=OP.mult)
            nc.vector.tensor_scalar(out=S2[:], in0=wE[:], scalar1=tk * tk,
                                    scalar2=None, op0=OP.mult)
        else:
            nc.gpsimd.tensor_tensor(out=S0[:], in0=S0[:], in1=wE[:],
                                    op=OP.add)
            nc.vector.scalar_tensor_tensor(
                out=S1[:], in0=wE[:], scalar=tk, in1=S1[:],
                op0=OP.mult, op1=OP.add)
            nc.vector.scalar_tensor_tensor(
                out=S2[:], in0=wE[:], scalar=tk * tk, in1=S2[:],
                op0=OP.mult, op1=OP.add)
    return S0, S1, S2


def build_bass():
    if "nc" in _BUILD_CACHE:
        return _BUILD_CACHE["nc"]
    _pin_act_table_set()
    nc = bacc.Bacc("TRN2", target_bir_lowering=False, debug=False,
                   num_devices=NCORES)
    dp = nc.declare_dram_parameter

    # --- kernel inputs (per-core host-prepped) ---
    lt = dp("lt", [128, NQ, NQ, 128], BF16, isOutput=False)
    dt = dp("dt", [128, NQ, NQ, 128], BF16, isOutput=False)
    xsb = dp("xsb", [128, 2048], BF16, isOutput=False)  # x_in^T 2 k-blocks
    cf32 = dp("cf32", [128, F_TOT], F32, isOutput=False)
    cbf16 = dp("cbf16", [128, B_TOT], BF16, isOutput=False)
    negA1 = dp("negA1", [128, NQ, H], BF16, isOutput=False)
    negA2 = dp("negA2", [128, NQ, HS], F32, isOutput=False)
    mgc = dp("mgc", [NN, MW], BF16, isOutput=False)
    ids = dp("ids", [128, NQ], I32, isOutput=False)

    c1o = dp("c1o", [128, NQ, H], F32, isOutput=True)  # full c1 (all cores)
    c2o = dp("c2o", [128, NQ, HS], F32, isOutput=True)  # per-core shard

    with tile.TileContext(nc) as tc:
        with tc.tile_pool(name="const", bufs=1) as cst, \
             tc.tile_pool(name="work", bufs=1) as wk, \
             tc.tile_pool(name="psum", bufs=4, space="PSUM") as psum, \
             tc.tile_pool(name="psmall", bufs=2, space="PSUM") as psmall, \
             tc.tile_pool(name="ptrp", bufs=2, space="PSUM") as ptrp:

            # ---------- grouped constant loads ----------
            # DMA bandwidth is ~358GB/s shared across queues, so order by
            # need: all small consts first at full rate, then lt, then dt.
            ids_sb = cst.tile([128, NQ], I32, tag="ids")
            nc.sync.dma_start(out=ids_sb[:], in_=ids[:])
            xs_sb = cst.tile([128, 2048], BF16, tag="xs")
            nc.sync.dma_start(out=xs_sb[:], in_=xsb[:])
            cb = cst.tile([128, B_TOT], BF16, tag="cb")
            nc.sync.dma_start(out=cb[:], in_=cbf16[:])
            cf = cst.tile([128, F_TOT], F32, tag="cf")
            nc.sync.dma_start(out=cf[:], in_=cf32[:])

            lt_sb = cst.tile([128, NQ, NQ, 128], BF16, tag="lt")
            dt_sb = cst.tile([128, NQ, NQ, 128], BF16, tag="dt")
            negA_sb = [cst.tile([128, NQ, H], BF16, tag="negA0",
                                name="negA_sb0"),
                       cst.tile([128, NQ, HS], F32, tag="negA1",
                                name="negA_sb1")]
            nc.scalar.dma_start(out=negA_sb[0][:], in_=negA1[:])
            nc.scalar.dma_start(out=negA_sb[1][:], in_=negA2[:])
            nc.sync.dma_start(out=lt_sb[:], in_=lt[:])
            nc.sync.dma_start(out=dt_sb[:], in_=dt[:])

            # const views
            btune_v = cf[:, F_BT:F_BT + 1]
            rms_v = [cf[:, F_R1:F_R1 + 1], cf[:, F_R2:F_R2 + 1]]
            bbc_v = [cf[:, F_BB1:F_BB1 + H + 1], cf[:, F_BB2:F_BB2 + HS + 1]]
            actb_v = cf[:, F_ACT:F_ACT + 9]
            ident_v = cf[:, F_ID:F_ID + 128]
            xs_v = [xs_sb[:, 0:1024], xs_sb[:, 1024:2048]]
            wt_v = [cb[:, B_WT0:B_WT0 + 128], cb[:, B_WT1:B_WT1 + 128]]
            wb_v = [cb[:, B_WB1:B_WB1 + H + 1], cb[:, B_WB2:B_WB2 + HS + 1]]
            ones_v = cb[:, B_ONE:B_ONE + 1]

            # memory-table gathers (software DGE on gpsimd; combined bf16
            # m1-full|m2-shard table: 1024 descriptors x 288B)
            mg = wk.tile([128, NQ, MW], BF16, tag="mg")
            for q in range(NQ):
                nc.gpsimd.indirect_dma_start(
                    out=mg[:, q, :],
                    out_offset=None,
                    in_=mgc[:],
                    in_offset=bass.IndirectOffsetOnAxis(
                        ap=ids_sb[:, q:q + 1], axis=0),
                )

            # zt^T = W_tune^T @ x_in^T + b_tune   [128 H, 1024 nodes] f32
            ztT = wk.tile([128, 1024], F32, tag="ztT")
            for hhalf in range(2):
                ps = psmall.tile([128, 512], F32, tag="sp")
                cols = slice(hhalf * 512, (hhalf + 1) * 512)
                nc.tensor.matmul(ps[:], lhsT=wt_v[0],
                                 rhs=xs_v[0][:, cols], start=True, stop=False)
                nc.tensor.matmul(ps[:], lhsT=wt_v[1],
                                 rhs=xs_v[1][:, cols], start=False, stop=True)
                nc.vector.tensor_scalar(out=ztT[:, cols], in0=ps[:],
                                        scalar1=btune_v, scalar2=None,
                                        op0=OP.add)

            u2T_full = wk.tile([128, 1024], F32, tag="u2T_full")
            ztp = wk.tile([128, NQ, H], F32, tag="ztp")
            c1g = 2.0 * 0.7978845608028654
            c2g = c1g * 0.044715

            couts = (c1o, c2o)

            for s, W in ((0, H), (1, HS)):  # replicated stage 1, sharded 2
                base = ztT if s == 0 else u2T_full

                # per-q front-end in transposed land (independent of W):
                # scaled bf16 lhsT + squares + row-sums; rinv per node-half
                baseS = wk.tile([128, 1024], BF16, tag=f"baseS{s}")
                sq = wk.tile([128, 1024], BF16, tag=f"sq{s}")
                ssp = wk.tile([128, NQ], F32, tag=f"ssp{s}")
                lnss = wk.tile([128, NQ], F32, tag=f"lnss{s}")
                rinv = wk.tile([128, NQ], F32, tag=f"rinv{s}")
                BD = wk.tile([128, NQ, W + 1], F32, tag=f"BD{s}")
                for h in range(4):
                    hcols = slice(h * 256, (h + 1) * 256)
                    nc.vector.tensor_scalar(out=baseS[:, hcols],
                                            in0=base[:, hcols],
                                            scalar1=rms_v[s], scalar2=None,
                                            op0=OP.mult)
                    nc.scalar.activation(sq[:, hcols], base[:, hcols],
                                         AF.Square)
                    for q in range(2 * h, 2 * h + 2):
                        cols = slice(q * 128, (q + 1) * 128)
                        ps = psmall.tile([128, 1], F32, tag="sp")
                        nc.tensor.matmul(ps[:], lhsT=sq[:, cols], rhs=ones_v,
                                         start=True, stop=True)
                        nc.scalar.activation(ssp[:, q:q + 1], ps[:], AF.Copy)
                    hq = slice(2 * h, 2 * h + 2)
                    nc.scalar.activation(lnss[:, hq], ssp[:, hq], AF.Ln)
                    nc.scalar.activation(rinv[:, hq], lnss[:, hq], AF.Exp,
                                         scale=-0.5, bias=actb_v[:, 0:1])
                    for q in range(2 * h, 2 * h + 2):
                        ps = psmall.tile([128, W + 1], F32, tag="sp")
                        nc.tensor.matmul(ps[:],
                                         lhsT=baseS[:, q * 128:(q + 1) * 128],
                                         rhs=wb_v[s], start=True, stop=True)
                        nc.vector.scalar_tensor_tensor(
                            out=BD[:, q, :], in0=ps[:], scalar=rinv[:, q:q + 1],
                            in1=bbc_v[s], op0=OP.mult, op1=OP.add)

                # delta = softplus(BD[...,W]) = ln(1+exp(x)), per quarter so
                # the chain gating the L1x pass ends 2 BD-stts after q=7
                esp = wk.tile([128, NQ, 1], F32, tag=f"esp{s}")
                ep1 = wk.tile([128, NQ, 1], F32, tag=f"ep1{s}")
                deltap = wk.tile([128, NQ, 1], F32, tag=f"deltap{s}")
                for h in range(4):
                    hq = slice(2 * h, 2 * h + 2)
                    nc.scalar.activation(esp[:, hq, :], BD[:, hq, W:W + 1],
                                         AF.Exp)
                    nc.vector.tensor_scalar(out=ep1[:, hq, :],
                                            in0=esp[:, hq, :], scalar1=1.0,
                                            scalar2=None, op0=OP.add)
                    nc.scalar.activation(deltap[:, hq, :], ep1[:, hq, :],
                                         AF.Ln)

                # rhs group tile R0 = [X | M | Y1 | V]; L1 reads [X|M],
                # D1 reads [M|Y1|V], D1x/L2 read [V]. R2 = [W1'|U'|UM']
                # (pre-scaled by REG^2 / -REG / -REG at the psum copies).
                R0 = wk.tile([128, NQ, 4 * W], BF16, tag=f"R0{s}")
                R2 = wk.tile([128, NQ, 3 * W], BF16, tag=f"R2{s}")
                acc = wk.tile([128, NQ, W], F32, tag=f"acc{s}")
                OUT2 = wk.tile([128, NQ, 3 * W], BF16, tag=f"OUT2{s}")

                # X = B*delta (bf16, straight into R0); dA = delta*negA
                # (bf16); At = exp(dA); M = m_gather*At
                for h in range(4):
                    hq = slice(2 * h, 2 * h + 2)
                    nc.vector.tensor_tensor(
                        out=R0[:, hq, 0:W], in0=BD[:, hq, 0:W],
                        in1=deltap[:, hq, :].to_broadcast([128, 2, W]),
                        op=OP.mult)
                dA = wk.tile([128, NQ, W], BF16, tag=f"dA{s}")
                nc.vector.tensor_tensor(
                    out=dA[:], in0=deltap[:].to_broadcast([128, NQ, W]),
                    in1=negA_sb[s][:], op=OP.mult)
                At = wk.tile([128, NQ, W], F32, tag=f"At{s}")
                nc.scalar.activation(At[:], dA[:], AF.Exp)
                Mf = wk.tile([128, NQ, W], F32, tag=f"Mf{s}")
                nc.gpsimd.tensor_tensor(out=Mf[:],
                                        in0=mg[:, :, s * H:s * H + W],
                                        in1=At[:], op=OP.mult)
                nc.vector.tensor_copy(out=R0[:, :, W:2 * W], in_=Mf[:])

                # zt packed [128 node-p, 8q, Wh] via PE transposes (stage 0)
                if s == 0:
                    for q in range(NQ):
                        pst = ptrp.tile([128, 128], F32, tag="trp")
                        nc.tensor.transpose(pst[:],
                                            ztT[:, q * 128:(q + 1) * 128],
                                            ident_v)
                        nc.vector.tensor_copy(out=ztp[:, q, :], in_=pst[:])

                def l1x_cb(q, ps, R0=R0, W=W):
                    # V = X - REG*LX  (bf16 into R0[3W:4W])
                    nc.vector.scalar_tensor_tensor(
                        out=R0[:, q, 3 * W:4 * W], in0=ps[:, 0:W], scalar=-REG,
                        in1=R0[:, q, 0:W], op0=OP.mult, op1=OP.add)

                def l2_cb(q, ps, R2=R2, W=W):
                    # W1' = REG^2 * L@V
                    nc.vector.tensor_scalar(out=R2[:, q, 0:W], in0=ps[:, 0:W],
                                            scalar1=REG2, scalar2=None,
                                            op0=OP.mult)

                def d2_cb(q, ps, OUT2=OUT2, W=W):
                    # ps = [P''| Q''| T2''] = [REG^2*P | -REG*Q | -REG*T2]
                    # OUT2 = [REG^2*P | REG^2/2*Q | REG^2/2*T2]
                    nc.scalar.activation(OUT2[:, q, 0:W], ps[:, 0:W], AF.Copy)
                    nc.vector.tensor_scalar(out=OUT2[:, q, W:3 * W],
                                            in0=ps[:, W:3 * W],
                                            scalar1=-REG / 2, scalar2=None,
                                            op0=OP.mult)

                def d1m_core(q, ps_um, ps_t1, R2=R2, acc=acc, Mf=Mf, W=W):
                    # UM' = -REG*UM; acc = Mf + UM' + REG^2*T1
                    nc.scalar.activation(R2[:, q, 2 * W:3 * W], ps_um,
                                         AF.Copy, scale=-REG)
                    nc.gpsimd.tensor_tensor(out=acc[:, q, :], in0=Mf[:, q, :],
                                            in1=R2[:, q, 2 * W:3 * W],
                                            op=OP.add)
                    nc.vector.scalar_tensor_tensor(
                        out=acc[:, q, :], in0=ps_t1, scalar=REG2,
                        in1=acc[:, q, :], op0=OP.mult, op1=OP.add)

                if s == 0:
                    # X-side passes first (gather-independent), M-side after
                    def d1x_cb(q, ps, R2=R2, W=W):
                        # U' = -REG * D@V
                        nc.scalar.activation(R2[:, q, W:2 * W], ps[:, 0:W],
                                             AF.Copy, scale=-REG)

                    def l1m_cb(q, ps, R0=R0, W=W):
                        nc.scalar.activation(R0[:, q, 2 * W:3 * W],
                                             ps[:, 0:W], AF.Copy)

                    def d1m_cb(q, ps, W=W):
                        # ps = [UM | T1]
                        d1m_core(q, ps[:, 0:W], ps[:, W:2 * W])

                    _heavy_pass(nc, psum, lt_sb, W, l1x_cb,
                                rhs_cols=lambda k: R0[:, k, 0:W])
                    _heavy_pass(nc, psum, dt_sb, W, d1x_cb,
                                rhs_cols=lambda k: R0[:, k, 3 * W:4 * W])
                    _heavy_pass(nc, psum, lt_sb, W, l2_cb,
                                rhs_cols=lambda k: R0[:, k, 3 * W:4 * W])
                    S0, S1, S2 = _moments(nc, wk, s, W, dA, actb_v)
                    _heavy_pass(nc, psum, lt_sb, W, l1m_cb,
                                rhs_cols=lambda k: R0[:, k, W:2 * W])
                    _heavy_pass(nc, psum, dt_sb, 2 * W, d1m_cb,
                                rhs_cols=lambda k: R0[:, k, W:3 * W])
                else:
                    # M available immediately: combined 4-pass structure
                    def l1_cb(q, ps, R0=R0, W=W):
                        nc.vector.scalar_tensor_tensor(
                            out=R0[:, q, 3 * W:4 * W], in0=ps[:, 0:W],
                            scalar=-REG, in1=R0[:, q, 0:W],
                            op0=OP.mult, op1=OP.add)
                        nc.scalar.activation(R0[:, q, 2 * W:3 * W],
                                             ps[:, W:2 * W], AF.Copy)

                    def d1_cb(q, ps, R2=R2, W=W):
                        # ps = [UM | T1 | U]  (rhs was [M|Y1|V])
                        nc.scalar.activation(R2[:, q, W:2 * W],
                                             ps[:, 2 * W:3 * W], AF.Copy,
                                             scale=-REG)
                        d1m_core(q, ps[:, 0:W], ps[:, W:2 * W])

                    _heavy_pass(nc, psum, lt_sb, 2 * W, l1_cb,
                                rhs_cols=lambda k: R0[:, k, 0:2 * W])
                    S0, S1, S2 = _moments(nc, wk, s, W, dA, actb_v)
                    _heavy_pass(nc, psum, dt_sb, 3 * W, d1_cb,
                                rhs_cols=lambda k: R0[:, k, W:4 * W])
                    _heavy_pass(nc, psum, lt_sb, W, l2_cb,
                                rhs_cols=lambda k: R0[:, k, 3 * W:4 * W])

                # ---- heavy pass D2: D @ [W1' | U' | UM'] -> P,Q,T2 ----
                # emitted per node-half, interleaved with the combine/gelu
                # tail so the tail ops aren't queued behind the later psum
                # drains on their engines
                tV = wk.tile([128, NQ, W], BF16, tag=f"tV{s}")
                tU = wk.tile([128, NQ, W], BF16, tag=f"tU{s}")
                tP = wk.tile([128, NQ, W], BF16, tag=f"tP{s}")
                tQ = wk.tile([128, NQ, W], BF16, tag=f"tQ{s}")
                if s == 0:
                    g1 = wk.tile([128, NQ, W], F32, tag="g1")
                    g2 = wk.tile([128, NQ, W], F32, tag="g2")
                    u2p = wk.tile([128, NQ, W], F32, tag="u2p")
                for h in range(4):
                    hq = slice(2 * h, 2 * h + 2)
                    _heavy_pass(nc, psum, dt_sb, 3 * W, d2_cb,
                                rhs_cols=lambda k: R2[:, k, :],
                                qs=range(2 * h, 2 * h + 2))
                    a = acc[:, hq, :]
                    nc.gpsimd.tensor_tensor(out=a, in0=a,
                                            in1=OUT2[:, hq, 2 * W:3 * W],
                                            op=OP.add)
                    nc.vector.tensor_tensor(out=tP[:, hq, :],
                                            in0=OUT2[:, hq, 0:W],
                                            in1=S1[:, hq, :], op=OP.mult)
                    nc.vector.tensor_tensor(out=tQ[:, hq, :],
                                            in0=OUT2[:, hq, W:2 * W],
                                            in1=S2[:, hq, :], op=OP.mult)
                    nc.vector.tensor_tensor(out=tV[:, hq, :],
                                            in0=R0[:, hq, 3 * W:4 * W],
                                            in1=S0[:, hq, :], op=OP.mult)
                    nc.vector.tensor_tensor(out=tU[:, hq, :],
                                            in0=R2[:, hq, W:2 * W],
                                            in1=S1[:, hq, :], op=OP.mult)
                    nc.vector.tensor_tensor(out=tP[:, hq, :], in0=tP[:, hq, :],
                                            in1=tQ[:, hq, :], op=OP.add)
                    nc.vector.tensor_tensor(out=tV[:, hq, :], in0=tV[:, hq, :],
                                            in1=tU[:, hq, :], op=OP.add)
                    nc.vector.tensor_tensor(out=tP[:, hq, :], in0=tP[:, hq, :],
                                            in1=tV[:, hq, :], op=OP.add)
                    nc.gpsimd.tensor_tensor(out=a, in0=a, in1=tP[:, hq, :],
                                            op=OP.add)
                    if s == 0:
                        # u2 = zt + gelu(c1) on the half; transpose to u2T
                        g1h, g2h, u2h = g1[:, hq, :], g2[:, hq, :], u2p[:, hq, :]
                        nc.gpsimd.tensor_tensor(out=g1h, in0=a, in1=a,
                                                op=OP.mult)
                        nc.vector.tensor_scalar(out=g1h, in0=g1h,
                                                scalar1=-c2g, scalar2=-c1g,
                                                op0=OP.mult, op1=OP.add)
                        nc.gpsimd.tensor_tensor(out=g2h, in0=a, in1=g1h,
                                                op=OP.mult)
                        nc.scalar.activation(g2h, g2h, AF.Exp)
                        nc.vector.tensor_scalar(out=g2h, in0=g2h, scalar1=1.0,
                                                scalar2=None, op0=OP.add)
                        nc.scalar.activation(g2h, g2h, AF.Ln)
                        nc.scalar.activation(g2h, g2h, AF.Exp, scale=-1.0)
                        nc.gpsimd.tensor_tensor(out=u2h, in0=a, in1=g2h,
                                                op=OP.mult)
                        nc.vector.tensor_tensor(out=u2h, in0=ztp[:, hq, :],
                                                in1=u2h, op=OP.add)
                        for q in range(2 * h, 2 * h + 2):
                            pst = ptrp.tile([128, 128], F32, tag="trp")
                            nc.tensor.transpose(pst[:], u2p[:, q, :], ident_v)
                            nc.vector.tensor_copy(
                                out=u2T_full[:, q * 128:(q + 1) * 128],
                                in_=pst[:])
                # write output (c1 full / c2 shard)
                nc.sync.dma_start(out=couts[s][:], in_=acc[:])

    nc.compile()
    _BUILD_CACHE["nc"] = nc
    return nc


def _pack_q(a_T):
    """[1024, 1024] transposed operator -> [128, 8q, 8k, 128] bf16,
    element [p, q, k, c] = a_T[k*128+p, q*128+c]."""
    r = a_T.reshape(NQ, 128, NQ, 128).transpose(1, 2, 0, 3)
    return np.ascontiguousarray(r).astype(BF)


def kernel(**inputs):
    out, _ = _run(inputs, trace=False)
    return out


def _run(inputs, trace=False, trace_kwargs=None):
    inp = {k: np.asarray(v) for k, v in inputs.items()}
    L = inp["L_agg"].astype(np.float32)
    D = inp["delta_L_agg"].astype(np.float32)
    x_sub = inp["x_sub"].astype(np.float32)
    m1 = inp["m1_vec"].astype(np.float32)
    m2 = inp["m2_vec"].astype(np.float32)
    names = inp["names_table"].astype(np.float32)
    rms1 = inp["rms1_scale"].astype(np.float32)
    rms2 = inp["rms2_scale"].astype(np.float32)
    W_tune = inp["W_tune"].astype(np.float32)
    b_tune = inp["b_tune"].astype(np.float32)
    W_B1 = inp["W_B1"].astype(np.float32)
    b_B1 = inp["b_B1"].astype(np.float32)
    W_B2 = inp["W_B2"].astype(np.float32)
    b_B2 = inp["b_B2"].astype(np.float32)
    W_dt = inp["W_dt"].astype(np.float32)
    b_dt = inp["b_dt"].astype(np.float32)
    A1 = inp["A_log_1"].astype(np.float32)
    A2 = inp["A_log_2"].astype(np.float32)
    tsrc = np.asarray(inp["target_src"]).astype(np.int64)
    tdst = np.asarray(inp["target_dst"]).astype(np.int64)
    aids = np.asarray(inp["active_input_ids"]).astype(np.int64)

    # x_in = [x_sub | neigh]; the names_table neighbor embedding (ED=1)
    neigh = np.zeros((NA, 2 * ED), np.float32)
    neigh[:E, :ED] = names[tsrc]
    neigh[:E, ED:] = names[tdst]
    neigh[E:2 * E, :ED] = names[tdst]
    neigh[E:2 * E, ED:] = names[tsrc]
    x_in = np.concatenate([x_sub, neigh], axis=1)  # [1024, 174]
    xsT = np.zeros((KDP, NA), np.float32)
    xsT[:KD] = x_in.T
    wtune_p = np.zeros((KDP, H), np.float32)
    wtune_p[:KD] = W_tune

    lt_p = _pack_q(np.ascontiguousarray(L.T))
    dt_p = _pack_q(np.ascontiguousarray(D.T))

    ids_p = np.ascontiguousarray(
        aids.astype(np.int32).reshape(NQ, 128).T)  # [128p, 8q]

    negA1_full = -np.exp(A1)  # [128]
    negA2_full = -np.exp(A2)

    eye = np.eye(128, dtype=np.float32)
    actb = np.array(
        [0.5 * np.log(H)] + [np.log(w) for w in T_W]
        + [np.log(w * t) for w, t in zip(T_W, T_NODES)], np.float32)

    xsb_np = np.concatenate([xsT[:128], xsT[128:]], axis=1).astype(BF)

    cbf = np.zeros((128, B_TOT), BF)
    cbf[:, B_WT0:B_WT0 + 128] = wtune_p[:128].astype(BF)
    cbf[:, B_WT1:B_WT1 + 128] = wtune_p[128:].astype(BF)
    cbf[:, B_WB1:B_WB1 + H + 1] = np.concatenate(
        [W_B1, W_dt], axis=1).astype(BF)
    cbf[:, B_ONE] = np.ones(128, BF)

    cff = np.zeros((128, F_TOT), np.float32)
    cff[:, F_BT] = b_tune
    cff[:, F_R1] = rms1
    cff[:, F_R2] = rms2
    cff[:, F_BB1:F_BB1 + H + 1] = np.concatenate([b_B1, b_dt])
    cff[:, F_ACT:F_ACT + 9] = actb
    cff[:, F_ID:F_ID + 128] = eye

    nA1 = np.tile(negA1_full, (128, NQ, 1)).astype(BF)
    m1bf = m1.astype(BF)

    common = {
        "lt": lt_p, "dt": dt_p, "ids": ids_p, "negA1": nA1, "xsb": xsb_np,
    }

    in_maps = []
    for c in range(NCORES):
        hs = slice(c * HS, (c + 1) * HS)
        cfc = cff.copy()
        cfc[:, F_BB2:F_BB2 + HS + 1] = np.concatenate([b_B2[hs], b_dt])
        cbc = cbf.copy()
        cbc[:, B_WB2:B_WB2 + HS + 1] = np.concatenate(
            [W_B2[:, hs], W_dt], axis=1).astype(BF)
        nA2 = np.tile(negA2_full[hs], (128, NQ, 1)).astype(np.float32)
        in_maps.append({
            **common,
            "cf32": cfc, "cbf16": cbc, "negA2": nA2,
            "mgc": np.ascontiguousarray(np.concatenate(
                [m1bf, m2[:, hs].astype(BF)], axis=1)),
        })

    nc = build_bass()
    res = run_bass_kernel_spmd(nc, in_maps, core_ids=list(range(NCORES)),
                               trace=trace, **(trace_kwargs or {}))

    out = np.zeros((2, NA, H), np.float32)
    # c1: full copy from core 0, packed [128p, 8q, 128h] -> [1024, 128]
    out[0] = res.results[0]["c1o"].transpose(1, 0, 2).reshape(NA, H)
    for c in range(NCORES):
        hs = slice(c * HS, (c + 1) * HS)
        out[1][:, hs] = res.results[c]["c2o"].transpose(1, 0, 2).reshape(NA, HS)
    return out, res
